# revision 1
# baseline (speedup 1.0000x reference)
"""DeepSeek-V3 MoE (T=4096, H=2048, E=32 top-8/32 grouped, I=1024, IS=2048)
on 8 trn2 NeuronCores — self-contained kernel.

Strategy (expert-parallel + token dispatch per the sharding hint):
- Routing (gate gemm + sigmoid + grouped top-k) runs on host in fp32: it is
  0.03%% of FLOPs, but expert SELECTION must match the fp32 reference exactly
  (device ACT sigmoid tables would flip near-tie selections and blow the
  absmax error); the resulting per-token combine weights are shipped to the
  device and folded into the expert intermediate activations.
- Core c owns experts 4c..4c+3. Tokens are gathered per expert into fixed
  capacity `cape` segments (512-blocks + optional 256 tail). All blocks of an
  i-tile accumulate in parallel PSUM banks, so expert weights stream from HBM
  exactly once per expert.
- The shared expert runs at full intermediate width over this core's T/8
  token slice (token-parallel: outputs disjoint, no partial-sum all-reduce).
- All gemms run on the PE in float32r (full-rate fp32, ~1.4e-4 rms error).
- Host scatters the dispatched outputs back and concatenates shared slices
  (the "all-reduce" of the weighted combine).
"""
import contextlib
import numpy as np

import concourse.bass as bass
import concourse.mybir as mybir
import concourse.tile as tile
from concourse import bacc

F32 = mybir.dt.float32
F32R = mybir.dt.float32r
AF = mybir.ActivationFunctionType

TOP_K, N_GROUP, TOPK_GROUP, ROUTED_SCALE = 8, 8, 4, 2.5
T, H, E, I, IS = 4096, 2048, 32, 1024, 2048
N_CORES = 8
EL = E // N_CORES          # 4 local experts
TSH = T // N_CORES         # 512-token shared slice per core
HT = H // 128              # 16
IT = I // 128              # 8
IST = IS // 128            # 16 i-tiles of the full shared intermediate
DEF_CAPE = 1280


def host_routing(x, gate_w, e_bias):
    """fp32 numpy replica of reference _routing_weights -> dense [T, E]."""
    logits = (x @ gate_w.T).astype(np.float32)
    scores = (1.0 / (1.0 + np.exp(-logits.astype(np.float32)))).astype(np.float32)
    swb = scores + e_bias[None, :].astype(np.float32)
    t, e = swb.shape
    gsz = e // N_GROUP
    grp = swb.reshape(t, N_GROUP, gsz)
    # top-2 sum per group (values only; ties irrelevant for a sum)
    top2 = np.sort(grp, axis=-1)[:, :, -2:]
    gscores = top2.sum(-1, dtype=np.float32)
    # top TOPK_GROUP groups, lowest-index-first on ties like jax.lax.top_k
    gidx = np.argsort(-gscores, axis=-1, kind="stable")[:, :TOPK_GROUP]
    gmask = np.zeros((t, N_GROUP), bool)
    np.put_along_axis(gmask, gidx, True, axis=1)
    emask = np.repeat(gmask, gsz, axis=1)
    masked = np.where(emask, swb, -np.inf)
    idx = np.argsort(-masked, axis=-1, kind="stable")[:, :TOP_K]
    w = np.take_along_axis(scores, idx, axis=1)
    w = (w / (w.sum(-1, keepdims=True) + 1e-20) * ROUTED_SCALE).astype(np.float32)
    wfull = np.zeros((t, e), np.float32)
    np.put_along_axis(wfull, idx, w, axis=1)
    return wfull



def blocks_of(cape):
    """[(offset, size), ...] covering cape with 512s + optional 256 tail."""
    out, off = [], 0
    while cape - off >= 512:
        out.append((off, 512))
        off += 512
    if cape - off:
        assert cape - off == 256, cape
        out.append((off, 256))
    return out


def build_nc2(cape=DEF_CAPE, repeat=1, mode="full", tsh=TSH):
    dma_only, pe_only = mode == "dma", mode == "pe"
    blks = blocks_of(cape)
    nc = bacc.Bacc("TRN2", target_bir_lowering=False)

    xs_d = nc.dram_tensor("xs", [H, EL * cape], F32R, kind="ExternalInput")
    xsh_d = nc.dram_tensor("xsh", [H, tsh], F32R, kind="ExternalInput")
    w13_d = nc.dram_tensor("w13", [EL, H, IT, 256], F32R, kind="ExternalInput")
    w2t_d = nc.dram_tensor("w2t", [EL, I, H], F32R, kind="ExternalInput")
    colw_d = nc.dram_tensor("colw", [128, EL * cape], F32, kind="ExternalInput")
    sw13_d = nc.dram_tensor("sw13", [H, IST, 256], F32R, kind="ExternalInput")
    sw2t_d = nc.dram_tensor("sw2t", [IS, H], F32R, kind="ExternalInput")
    yd_d = nc.dram_tensor("yd", [H, EL * cape], F32, kind="ExternalOutput")
    ys_d = nc.dram_tensor("ys", [H, tsh], F32, kind="ExternalOutput")

    with tile.TileContext(nc) as tc:
        with (
            tc.tile_pool(name="xp", bufs=1) as xp,
            tc.tile_pool(name="wp", bufs=6) as wp,
            tc.tile_pool(name="w2p", bufs=1) as w2p,
            tc.tile_pool(name="hp", bufs=1) as hp,
            tc.tile_pool(name="cp", bufs=2) as cp,
            tc.tile_pool(name="sp", bufs=1) as sp,
            tc.tile_pool(name="ps", bufs=1, space="PSUM") as ps,
        ):
            static_w = {}
            if pe_only:
                w13st = wp.tile([128, 256], F32R, tag="w13s", name="w13st")
                nc.sync.dma_start(w13st[:], w13_d[0, 0:128, 0, :])
                w2st = w2p.tile([128, 256], F32R, tag="w2s", name="w2st",
                                bufs=1)
                nc.sync.dma_start(w2st[:], w2t_d[0, 0:128, 0:256])
                static_w = {"w13": w13st, "w2": w2st}
            rep = tc.For_i(0, repeat, 1) if repeat > 1 else contextlib.nullcontext()
            with rep:
                # ============ routed experts over dispatched tokens
                for j in range(EL):
                    s0 = j * cape
                    x_sb = xp.tile([128, HT * cape], F32R, tag="x",
                                   name="x_sb")
                    for h in range(HT):
                        nc.sync.dma_start(
                            x_sb[:, bass.ts(h, cape)],
                            xs_d[128 * h : 128 * (h + 1), s0 : s0 + cape])
                    col_sb = cp.tile([128, cape], F32, tag="col",
                                     name="col_sb")
                    nc.sync.dma_start(col_sb[:], colw_d[:, s0 : s0 + cape])

                    h_sb = hp.tile([128, IT * cape], F32R, tag="h",
                                   name="h_sb")
                    # ---- phase C: h = silu(w1@x) * (w3@x) * colw
                    for ig in range(IT):
                        if not dma_only:
                            g_ps = [ps.tile([128, bs], F32, tag=f"g{b}",
                                            name=f"g_ps{b}")
                                    for b, (_, bs) in enumerate(blks)]
                            u_ps = [ps.tile([128, bs], F32, tag=f"u{b}",
                                            name=f"u_ps{b}")
                                    for b, (_, bs) in enumerate(blks)]
                        for h in range(HT):
                            if pe_only:
                                w13_sb = static_w["w13"]
                            else:
                                w13_sb = wp.tile([128, 256], F32R, tag="w13",
                                                 name="w13_sb")
                                nc.sync.dma_start(
                                    w13_sb[:],
                                    w13_d[j, 128 * h : 128 * (h + 1), ig, :])
                            if dma_only:
                                continue
                            for b, (bo, bs) in enumerate(blks):
                                rhs = x_sb[:, h * cape + bo : h * cape + bo + bs]
                                nc.tensor.matmul(
                                    g_ps[b][:], w13_sb[:, 0:128], rhs,
                                    start=(h == 0), stop=(h == HT - 1))
                                nc.tensor.matmul(
                                    u_ps[b][:], w13_sb[:, 128:256], rhs,
                                    start=(h == 0), stop=(h == HT - 1))
                        if dma_only:
                            continue
                        for b, (bo, bs) in enumerate(blks):
                            silu_sb = sp.tile([128, 512], F32, tag="silu",
                                              bufs=4, name="silu_sb")
                            nc.scalar.activation(silu_sb[:, :bs], g_ps[b][:],
                                                 AF.Silu)
                            hsl = h_sb[:, ig * cape + bo : ig * cape + bo + bs]
                            nc.vector.tensor_mul(hsl, u_ps[b][:],
                                                 col_sb[:, bo : bo + bs])
                            nc.vector.tensor_mul(hsl, hsl, silu_sb[:, :bs])

                    # ---- phase D: yd = w2 @ h
                    for hg in range(HT // 2):
                        w2_sb = []
                        for i in range(IT):
                            if pe_only:
                                w2_sb.append(static_w["w2"])
                                continue
                            w2c = w2p.tile([128, 256], F32R, tag="w2", bufs=24,
                                           name="w2c")
                            nc.sync.dma_start(
                                w2c[:],
                                w2t_d[j, 128 * i : 128 * (i + 1),
                                      256 * hg : 256 * (hg + 1)])
                            w2_sb.append(w2c)
                        for hl in range(2):
                            hrow = 256 * hg + 128 * hl
                            for b, (bo, bs) in enumerate(blks):
                                if dma_only:
                                    nc.sync.dma_start(
                                        yd_d[hrow : hrow + 128,
                                             s0 + bo : s0 + bo + bs],
                                        x_sb[:, bo : bo + bs].bitcast(F32))
                                    continue
                                o_ps = ps.tile([128, bs], F32, tag="o", bufs=2,
                                               name="o_ps")
                                for i in range(IT):
                                    nc.tensor.matmul(
                                        o_ps[:],
                                        w2_sb[i][:, bass.ts(hl, 128)],
                                        h_sb[:, i * cape + bo
                                             : i * cape + bo + bs],
                                        start=(i == 0), stop=(i == IT - 1))
                                ost = sp.tile([128, 512], F32, tag="ost",
                                              bufs=4, name="ost")
                                nc.vector.tensor_copy(ost[:, :bs], o_ps[:])
                                nc.sync.dma_start(
                                    yd_d[hrow : hrow + 128,
                                         s0 + bo : s0 + bo + bs],
                                    ost[:, :bs])

                # ============ shared expert, full IS, this core's 512 tokens
                x_sb = xp.tile([128, HT * tsh], F32R, tag="x", name="xsh_sb")
                for h in range(HT):
                    nc.sync.dma_start(x_sb[:, bass.ts(h, tsh)],
                                      xsh_d[128 * h : 128 * (h + 1), :])
                hs_sb = hp.tile([128, IST * tsh], F32R, tag="h", name="hs_sb")
                for ig in range(IST):
                    if not dma_only:
                        g_ps = ps.tile([128, tsh], F32, tag=f"g{ig % 2}",
                                       name="g_ps0")
                        u_ps = ps.tile([128, tsh], F32, tag=f"u{ig % 2}",
                                       name="u_ps0")
                    for h in range(HT):
                        if pe_only:
                            w13_sb = static_w["w13"]
                        else:
                            w13_sb = wp.tile([128, 256], F32R, tag="w13",
                                             name="w13_sb")
                            nc.sync.dma_start(
                                w13_sb[:],
                                sw13_d[128 * h : 128 * (h + 1), ig, :])
                        if dma_only:
                            continue
                        rhs = x_sb[:, bass.ts(h, tsh)]
                        nc.tensor.matmul(g_ps[:], w13_sb[:, 0:128], rhs,
                                         start=(h == 0), stop=(h == HT - 1))
                        nc.tensor.matmul(u_ps[:], w13_sb[:, 128:256], rhs,
                                         start=(h == 0), stop=(h == HT - 1))
                    if dma_only:
                        continue
                    silu_sb = sp.tile([128, 512], F32, tag="silu", bufs=4,
                                      name="silu_sb")
                    nc.scalar.activation(silu_sb[:, :tsh], g_ps[:], AF.Silu)
                    hsl = hs_sb[:, bass.ts(ig, tsh)]
                    nc.vector.tensor_mul(hsl, u_ps[:], silu_sb[:, :tsh])
                for hg in range(HT // 2):
                    w2_sb = []
                    for i in range(IST):
                        if pe_only:
                            w2_sb.append(static_w["w2"])
                            continue
                        w2c = w2p.tile([128, 256], F32R, tag="w2", bufs=24,
                                       name="w2c")
                        nc.sync.dma_start(
                            w2c[:],
                            sw2t_d[128 * i : 128 * (i + 1),
                                   256 * hg : 256 * (hg + 1)])
                        w2_sb.append(w2c)
                    for hl in range(2):
                        hrow = 256 * hg + 128 * hl
                        if dma_only:
                            nc.sync.dma_start(ys_d[hrow : hrow + 128, :],
                                              x_sb[:, 0:tsh].bitcast(F32))
                            continue
                        o_ps = ps.tile([128, tsh], F32, tag="o", bufs=2,
                                       name="o_ps")
                        for i in range(IST):
                            nc.tensor.matmul(
                                o_ps[:], w2_sb[i][:, bass.ts(hl, 128)],
                                hs_sb[:, bass.ts(i, tsh)],
                                start=(i == 0), stop=(i == IST - 1))
                        ost = sp.tile([128, 512], F32, tag="ost", bufs=4,
                                      name="ost")
                        nc.vector.tensor_copy(ost[:, :tsh], o_ps[:])
                        nc.sync.dma_start(ys_d[hrow : hrow + 128, :],
                                          ost[:, :tsh])
    nc.compile()
    return nc


def prep_inputs2(hidden_states, gate_w, e_bias, w1, w3, w2, sw1, sw3, sw2,
                 cape=DEF_CAPE):
    x = np.asarray(hidden_states, np.float32)
    t_total = x.shape[0]
    tsh = t_total // N_CORES
    xT = np.ascontiguousarray(x.T)
    wfull = host_routing(x, np.asarray(gate_w, np.float32),
                         np.asarray(e_bias, np.float32))

    w1t = np.asarray(w1, np.float32).transpose(0, 2, 1)
    w3t = np.asarray(w3, np.float32).transpose(0, 2, 1)
    ne = w1t.shape[0]
    w13 = np.empty((ne, H, IT, 256), np.float32)
    w13[..., 0:128] = np.ascontiguousarray(w1t).reshape(ne, H, IT, 128)
    w13[..., 128:256] = np.ascontiguousarray(w3t).reshape(ne, H, IT, 128)
    w2t = np.ascontiguousarray(np.asarray(w2, np.float32).transpose(0, 2, 1))

    sw1t = np.ascontiguousarray(np.asarray(sw1, np.float32).T)  # [H, IS]
    sw3t = np.ascontiguousarray(np.asarray(sw3, np.float32).T)
    sw13 = np.empty((H, IST, 256), np.float32)
    for ig in range(IST):
        sw13[:, ig, 0:128] = sw1t[:, 128 * ig : 128 * (ig + 1)]
        sw13[:, ig, 128:256] = sw3t[:, 128 * ig : 128 * (ig + 1)]
    sw2t = np.ascontiguousarray(np.asarray(sw2, np.float32).T)  # [IS, H]

    in_maps, scat = [], []
    for c in range(N_CORES):
        el0 = c * EL
        idxs, toks, colv = [], [], []
        for j in range(EL):
            idx = np.nonzero(wfull[:, el0 + j])[0]
            assert len(idx) <= cape, f"expert {el0+j}: {len(idx)} > {cape}"
            pad = cape - len(idx)
            toks.append(np.concatenate([idx, np.zeros(pad, np.int64)]))
            colv.append(np.concatenate([wfull[idx, el0 + j],
                                        np.zeros(pad, np.float32)]))
            idxs.append(idx)
        tok_list = np.concatenate(toks)
        colv = np.concatenate(colv).astype(np.float32)
        in_maps.append({
            "xs": np.ascontiguousarray(xT[:, tok_list]),
            "xsh": np.ascontiguousarray(xT[:, c * tsh : (c + 1) * tsh]),
            "w13": np.ascontiguousarray(w13[el0 : el0 + EL]),
            "w2t": np.ascontiguousarray(w2t[el0 : el0 + EL]),
            "colw": np.ascontiguousarray(
                np.broadcast_to(colv[None, :], (128, EL * cape))),
            "sw13": sw13, "sw2t": sw2t,
        })
        scat.append(idxs)
    return in_maps, scat


def combine2(results, scat, t_total=T, cape=DEF_CAPE):
    tsh = t_total // N_CORES
    acc = np.zeros((H, t_total), np.float32)
    for c in range(N_CORES):
        acc[:, c * tsh : (c + 1) * tsh] = results[c]["ys"]
    for c in range(N_CORES):
        yd = results[c]["yd"]
        for j, idx in enumerate(scat[c]):
            acc[:, idx] += yd[:, j * cape : j * cape + len(idx)]
    return np.ascontiguousarray(acc.T)


def pick_cape(counts):
    m = int(counts.max())
    cape = ((m + 255) // 256) * 256
    if cape % 512 == 256 and cape < 512:
        cape = 512
    return max(cape, 512)


_NC_CACHE = {}


def run2(inputs, cape=None):
    from concourse.bass_utils import run_bass_kernel_spmd
    x = np.asarray(inputs["hidden_states"], np.float32)
    if cape is None:
        wfull = host_routing(x, np.asarray(inputs["gate_w"], np.float32),
                             np.asarray(inputs["e_bias"], np.float32))
        cape = max(DEF_CAPE, pick_cape((wfull != 0).sum(0)))
    key = (cape, x.shape[0])
    if key not in _NC_CACHE:
        _NC_CACHE[key] = build_nc2(cape=cape, tsh=x.shape[0] // N_CORES)
    nc = _NC_CACHE[key]
    in_maps, scat = prep_inputs2(**inputs, cape=cape)
    res = run_bass_kernel_spmd(nc, in_maps, core_ids=list(range(N_CORES)))
    return combine2(res.results, scat, t_total=x.shape[0], cape=cape), res


def kernel(**inputs) -> np.ndarray:
    out, _ = run2(inputs)
    return np.asarray(out, np.float32)

